# revision 19
# baseline (speedup 1.0000x reference)
"""GAT layer (N=8192, F_IN=256, H=64 per head, K=8 heads) on 8 Trainium2 cores.

Strategy (row-sharding, fully data-parallel, no collectives):
  reference per head k:
    h   = features @ W[k]                      [N, H]
    wh1 = h @ a[k,:H]; wh2 = h @ a[k,H:]       [N]
    e   = leaky_relu(wh1[:,None] + wh2[None,:], 0.2)
    att = softmax(where(adj>0, e, -9e15), axis=1)
    out = elu(att @ h)

  Algebra: with s = wh1[i] + wh2[j],
    exp(lrelu(s)) = e1_i * e2_j * max(G1_i * G2_j, 1)
  where G1 = exp(0.8 wh1), G2 = exp(0.8 wh2), e2 = exp(0.2 wh2); the row
  factor e1_i cancels in softmax.  Key identity used on device:
    max(G1_i G2_j, 1) * e2_j = (G2_j e2_j) * max(G1_i, 1/G2_j)
  so with the per-column factor G2 e2 folded into the value matrix
  (host-precomputed stationary wst = [h * G2e2 | G2e2], streamed from HBM
  per chunk), the masked unnormalized attention tile is
    u[j,i] = max(g1b[i], s[j]) * adj[i,j]
  built per (head, g) on a statically assigned engine path:
    'w'  z = max(g1,s) on DVE ts; mask via one batched DVE tensor_tensor
         (adj broadcast across head slots)
    'wg' z on DVE ts; mask on GPSIMD tensor_tensor
    'a'  z = Relu(g1b - s[j]) on ACT; mask on DVE; PE correction matmuls
         acc += wcor @ adj (wcor = [h e2 | e2]) restore the +s[j] term
  The assignment (n_a=248 / n_wg=165 / n_w=99) balances DVE / ACT /
  GPSIMD / PE.  out[i] = elu(num/den) from transposed PSUM accumulators.

Per-core layout ([j,i]-transposed tiles so contraction j sits on partitions):
  adjr [128, 64, 2, 512] bf16 : adjr[p,g,b,i] = adj[r0+b*512+i, g*128+p]
  g1b  [128, 8, 1024] bf16    : G1 row broadcast across partitions
  sst  [128, 8, 64] f32       : sst[p,k,g] = 1/G2[g*128+p, k]
  wst  [128, 64, 8, 65] bf16  : [h*G2e2 | G2e2] stationaries
  wcor [128, 64, 4, 65] bf16  : [h*e2 | e2] correction stationaries (act heads)
"""

import sys
import os

sys.path.insert(0, "/opt/trn_rl_repo")

import numpy as np
import ml_dtypes
from contextlib import ExitStack

import concourse.bass as bass
import concourse.tile as tile
from concourse import bacc, mybir
from concourse.bass_utils import run_bass_kernel_spmd

N = 8192
F_IN = 256
H = 64
K = 8
ALPHA = 0.2
N_CORES = 8
R = N // N_CORES          # 1024 rows per core
IB = 2                    # i-blocks per core (512 columns of out-rows each)
IW = R // IB              # 512
G = N // 128              # 64 j-groups of 128
G_SUB = 8                 # j-groups per adjacency DMA
HA = H + 1                # 65: head value columns + denominator column
ACT_HEADS = (0, 1, 2, 3)  # heads with correction stationaries resident

F32 = mybir.dt.float32
BF16 = mybir.dt.bfloat16
AX = mybir.AluOpType

_cached = {}


def default_assign(n_a=248, n_wg=165):
    """Path per (k, g):
      'a'  ACT z=relu(g1-s) + DVE mask + PE correction matmuls
      'wg' DVE ts z=max(g1,s) + GPSIMD mask
      'w'  DVE ts + DVE mask
    HW-calibrated balance: a~248, wg~165, w~99 puts DVE/ACT/GP/PE all
    near ~345us (sim)."""
    table = {}
    # heads 0..3 are the act-capable ones; drop (256 - n_a) of them to 'w'
    n_drop = 4 * G - n_a
    drops = {(k, (k * G) // 4 + (i * G) // max(n_drop // 4, 1) % G)
             for k in range(4) for i in range(n_drop // 4)}
    for k in range(4):
        dropped = 0
        for g in range(G):
            if dropped < n_drop // 4 and (g * (n_drop // 4 + 1)) // G > (
                    (g - 1) * (n_drop // 4 + 1)) // G and g > 0:
                table[(k, g)] = "w"
                dropped += 1
            else:
                table[(k, g)] = "a"
    # heads 4..7: spread n_wg 'wg' Bresenham-style, rest 'w'
    per = [n_wg // 4 + (1 if i < n_wg % 4 else 0) for i in range(4)]
    for i, k in enumerate(range(4, 8)):
        q = per[i]
        for g in range(G):
            gq = (g + i * 16) % G  # rotate start per head
            table[(k, g)] = "wg" if (gq * q) // G != ((gq - 1) * q) // G or (
                gq == 0 and q > 0) else "w"
    return table


def build_program(loop_t=1, assign=None):
    key = (loop_t, tuple(sorted(assign.items())) if assign else None)
    if key in _cached:
        return _cached[key]
    table = assign or default_assign()

    nc = bacc.Bacc("TRN2", target_bir_lowering=False, debug=False,
                   num_devices=N_CORES)

    adjr_d = nc.dram_tensor("adjr", [128, G, IB, IW], BF16, kind="ExternalInput").ap()
    g1b_d = nc.dram_tensor("g1b", [128, K, IB * IW], BF16, kind="ExternalInput").ap()
    sst_d = nc.dram_tensor("sst", [128, K, G], F32, kind="ExternalInput").ap()
    nsst_d = nc.dram_tensor("nsst", [128, K, G], F32, kind="ExternalInput").ap()
    wst_d = nc.dram_tensor("wst", [128, G, K, HA], BF16, kind="ExternalInput").ap()
    wcor_d = nc.dram_tensor("wcor", [128, G, len(ACT_HEADS), HA], BF16,
                            kind="ExternalInput").ap()
    out_d = nc.dram_tensor("out", [R, K * H], F32, kind="ExternalOutput").ap()

    with tile.TileContext(nc) as tc:
        with ExitStack() as ctx:
            const = ctx.enter_context(tc.tile_pool(name="const", bufs=1))
            adj_pool = ctx.enter_context(tc.tile_pool(name="adj", bufs=2))
            u_pool = ctx.enter_context(tc.tile_pool(name="u", bufs=6))
            z_pool = ctx.enter_context(tc.tile_pool(name="z", bufs=6))
            wst_pool = ctx.enter_context(tc.tile_pool(name="wst", bufs=2))
            wcor_pool = ctx.enter_context(tc.tile_pool(name="wcor", bufs=2))
            stg_pool = ctx.enter_context(tc.tile_pool(name="stg", bufs=4))
            fin_pool = ctx.enter_context(tc.tile_pool(name="fin", bufs=2))
            acc_sb_pool = ctx.enter_context(tc.tile_pool(name="accsb", bufs=2))
            psum = ctx.enter_context(tc.tile_pool(name="psum", bufs=8, space="PSUM"))

            # ---- constants (outside the timing loop) ----
            g1b_sb = const.tile([128, K, IB * IW], BF16)
            nc.sync.dma_start(g1b_sb[:], g1b_d[:])
            sst_sb = const.tile([128, K, G], F32)
            nc.sync.dma_start(sst_sb[:], sst_d[:])
            nsst_sb = const.tile([128, K, G], F32)
            nc.sync.dma_start(nsst_sb[:], nsst_d[:])
            ident = const.tile([128, 128], F32)
            from concourse.masks import make_identity
            make_identity(nc, ident[:])

            loop_cm = tc.For_i(0, loop_t, 1) if loop_t > 1 else None
            if loop_cm is not None:
                ctx.enter_context(loop_cm)

            PAIR = IB * IW  # 1024

            for sweep_heads in ((0, 1, 4, 5), (2, 3, 6, 7)):
                accs = {}
                for k in sweep_heads:
                    for ib in range(IB):
                        accs[(k, ib)] = psum.tile(
                            [HA, IW], F32, tag="ps", name=f"acc{k}_{ib}")
                for gs in range(G // G_SUB):
                    adj_t = adj_pool.tile([128, G_SUB, IB, IW], BF16)
                    nc.sync.dma_start(
                        adj_t[:], adjr_d[:, gs * G_SUB:(gs + 1) * G_SUB, :, :]
                    )
                    wst_t = wst_pool.tile([128, G_SUB, K, HA], BF16)
                    nc.sync.dma_start(
                        wst_t[:], wst_d[:, gs * G_SUB:(gs + 1) * G_SUB, :, :]
                    )
                    wcor_t = wcor_pool.tile([128, G_SUB, len(ACT_HEADS), HA],
                                            BF16)
                    nc.sync.dma_start(
                        wcor_t[:],
                        wcor_d[:, gs * G_SUB:(gs + 1) * G_SUB, :, :]
                    )
                    for gi in range(G_SUB):
                        g = gs * G_SUB + gi
                        adj_pair = adj_t[:, gi, :, :].rearrange("p b i -> p (b i)")
                        # slot order: DVE-masked heads ('w','a') first so one
                        # broadcast tensor_tensor masks them all, 'wg' last
                        order = {"w": 0, "a": 1, "wg": 2}
                        heads = sorted(sweep_heads,
                                       key=lambda k: order[table[(k, g)]])
                        n_dve = sum(1 for k in heads if table[(k, g)] != "wg")
                        z_all = z_pool.tile([128, 4, PAIR], BF16, tag="z")
                        u_all = u_pool.tile([128, 4, PAIR], BF16, tag="u")
                        for si, k in enumerate(heads):
                            path = table[(k, g)]
                            g1 = g1b_sb[:, k, :]
                            if path == "a":
                                nc.scalar.activation(
                                    z_all[:, si, :], g1,
                                    mybir.ActivationFunctionType.Relu,
                                    bias=nsst_sb[:, k, g:g + 1],
                                )
                            else:
                                nc.vector.tensor_scalar(
                                    z_all[:, si, :], g1,
                                    sst_sb[:, k, g:g + 1], None, op0=AX.max)
                        if n_dve:
                            nc.vector.tensor_tensor(
                                u_all[:, 0:n_dve, :], z_all[:, 0:n_dve, :],
                                adj_pair.unsqueeze(1)
                                .broadcast_to((128, n_dve, PAIR)),
                                op=AX.mult)
                        for si in range(n_dve, 4):
                            nc.gpsimd.tensor_tensor(
                                u_all[:, si, :], z_all[:, si, :], adj_pair,
                                op=AX.mult)
                        for si, k in enumerate(heads):
                            path = table[(k, g)]
                            for ib in range(IB):
                                nc.tensor.matmul(
                                    accs[(k, ib)][:],
                                    wst_t[:, gi, k, :],
                                    u_all[:, si, ib * IW:(ib + 1) * IW],
                                    start=(g == 0),
                                    stop=(g == G - 1 and path != "a"),
                                )
                            if path == "a":
                                ci = ACT_HEADS.index(k)
                                for ib in range(IB):
                                    nc.tensor.matmul(
                                        accs[(k, ib)][:],
                                        wcor_t[:, gi, ci, :],
                                        adj_t[:, gi, ib, :],
                                        start=False,
                                        stop=(g == G - 1),
                                    )
                # ---- drain this sweep ----
                sw_sorted = sorted(sweep_heads)
                runs = []
                for k in sw_sorted:
                    if runs and runs[-1][-1] == k - 1:
                        runs[-1].append(k)
                    else:
                        runs.append([k])
                for ib in range(IB):
                    stgs = [stg_pool.tile([128, 4, HA], F32, tag="stg",
                                          name=f"stg{sweep_heads[0]}_{ib}_{c}")
                            for c in range(IW // 128)]
                    for si, k in enumerate(sw_sorted):
                        acc_sb = acc_sb_pool.tile([HA, IW], F32, tag="accsb")
                        nc.scalar.copy(acc_sb[:], accs[(k, ib)][:])
                        for c in range(IW // 128):
                            pst = psum.tile([128, HA], F32, tag="ps",
                                            name=f"pst{k}_{ib}_{c}")
                            nc.tensor.transpose(
                                pst[:], acc_sb[:, c * 128:(c + 1) * 128],
                                ident[0:HA, 0:HA],
                            )
                            nc.scalar.copy(stgs[c][:, si, :], pst[:])
                    for c in range(IW // 128):
                        stg = stgs[c]
                        recips = fin_pool.tile([128, 4], F32, tag="recip")
                        nc.vector.reciprocal(recips[:], stg[:, :, H])
                        fin = fin_pool.tile([128, 4, H], F32, tag="fin")
                        nc.vector.tensor_tensor(
                            fin[:], stg[:, :, 0:H],
                            recips[:].unsqueeze(2).broadcast_to((128, 4, H)),
                            op=AX.mult,
                        )
                        # elu(x) = exp(min(x,0)) + (max(x,0) - 1)
                        fin2 = fin_pool.tile([128, 4 * H], F32, tag="fin2")
                        finf = fin[:].rearrange("p k f -> p (k f)")
                        nc.vector.tensor_scalar(
                            fin2[:], finf, 0.0, None, op0=AX.min
                        )
                        ex = fin_pool.tile([128, 4 * H], F32, tag="ex")
                        nc.scalar.activation(
                            ex[:], fin2[:], mybir.ActivationFunctionType.Exp
                        )
                        rel = fin_pool.tile([128, 4 * H], F32, tag="rel")
                        nc.vector.tensor_scalar(
                            rel[:], finf, 0.0, -1.0, op0=AX.max, op1=AX.add
                        )
                        res = fin_pool.tile([128, 4 * H], F32, tag="res")
                        nc.vector.tensor_tensor(res[:], ex[:], rel[:], op=AX.add)
                        resv = res[:].rearrange("p (k f) -> p k f", k=4)
                        for run in runs:
                            s0 = sw_sorted.index(run[0])
                            nc.sync.dma_start(
                                out_d[ib * IW + c * 128:
                                      ib * IW + (c + 1) * 128,
                                      run[0] * H:(run[-1] + 1) * H],
                                resv[:, s0:s0 + len(run), :],
                            )

    nc.compile()
    _cached[key] = nc
    return nc


def prepare_inputs(features, adj, W, a):
    """Host-side prep: O(N*F*K) projections + per-core sharded layouts."""
    features = np.asarray(features, dtype=np.float32)
    adj = np.asarray(adj, dtype=np.float32)
    W = np.asarray(W, dtype=np.float32)
    a = np.asarray(a, dtype=np.float32)

    h_all = np.einsum("nf,kfh->knh", features, W)        # [K, N, H]
    wh1 = np.einsum("knh,kh->nk", h_all, a[:, :H])       # [N, K]
    wh2 = np.einsum("knh,kh->nk", h_all, a[:, H:])       # [N, K]
    G1 = np.exp(0.8 * wh1).astype(np.float32)
    G2 = np.exp(0.8 * wh2).astype(np.float32)
    E25 = np.exp(0.2 * wh2).astype(np.float32)
    Sst = (1.0 / G2).astype(np.float32)                  # [N, K]
    G2E = (G2 * E25).astype(np.float32)

    # wst[p, g, k, :] = [h[j]*G2e2[j], G2e2[j]] at j = g*128+p
    wst = np.empty((128, G, K, HA), dtype=np.float32)
    wcor = np.empty((128, G, len(ACT_HEADS), HA), dtype=np.float32)
    hj = h_all.transpose(1, 0, 2)                        # [N, K, H]
    for g in range(G):
        j0 = g * 128
        blk = hj[j0:j0 + 128]                            # [128, K, H]
        wst[:, g, :, 0:H] = blk * G2E[j0:j0 + 128, :, None]
        wst[:, g, :, H] = G2E[j0:j0 + 128, :]
        wcor[:, g, :, 0:H] = (blk[:, ACT_HEADS, :]
                              * E25[j0:j0 + 128][:, ACT_HEADS, None])
        wcor[:, g, :, H] = E25[j0:j0 + 128][:, ACT_HEADS]
    wst = wst.astype(ml_dtypes.bfloat16)
    wcor = wcor.astype(ml_dtypes.bfloat16)

    # sst[p, k, g] = 1/G2[g*128+p, k]
    sst = np.ascontiguousarray(
        Sst.reshape(G, 128, K).transpose(1, 2, 0))       # [128, K, G]

    in_maps = []
    for c in range(N_CORES):
        r0 = c * R
        blk = adj[r0:r0 + R, :]                          # [R, N]
        adj_r = np.ascontiguousarray(
            blk.reshape(IB, IW, G, 128).transpose(3, 2, 0, 1)
        ).astype(ml_dtypes.bfloat16)                     # [128, G, IB, IW]
        g1_blk = G1[r0:r0 + R, :].T                      # [K, R]
        g1b = np.broadcast_to(
            g1_blk[None].astype(ml_dtypes.bfloat16), (128, K, R))
        g1b = np.ascontiguousarray(g1b)
        in_maps.append({
            "adjr": adj_r,
            "g1b": g1b,
            "sst": sst,
            "nsst": -sst,
            "wst": wst,
            "wcor": wcor,
        })
    return in_maps


def kernel(features, adj, W, a):
    nc = build_program()
    in_maps = prepare_inputs(features, adj, W, a)
    res = run_bass_kernel_spmd(nc, in_maps, list(range(N_CORES)))
    out = np.concatenate(
        [res.results[c]["out"] for c in range(N_CORES)], axis=0)
    return out.astype(np.float32)


if __name__ == "__main__":
    rng = np.random.default_rng(0)
    features = rng.standard_normal((N, F_IN), dtype=np.float32)
    adj = (rng.integers(0, 2, size=(N, N))).astype(np.float32)
    W = (rng.standard_normal((K, F_IN, H), dtype=np.float32) * 0.118)
    a = (rng.standard_normal((K, 2 * H), dtype=np.float32) * 0.176)
    out = kernel(features=features, adj=adj, W=W, a=a)
    print("out", out.shape, out.dtype, np.abs(out).max())
